# revision 54
# baseline (speedup 1.0000x reference)
"""LocalGlobalAttention Bass/Tile kernel for 8 Trainium2 NeuronCores.

Sharding: data-parallel over (batch=2) x (4 sequence chunks of 512).
Each core independently computes QKV projection (with +-32 token halo),
banded local attention (|i-j|<=32), global attention over tokens 0..3,
and the output projection for its 512-token slice. No collectives.

Exact-math host-side folds (same as v1):
 - top_k(softmax(g), 4) over a length-4 g selects all 4 indices ->
   global attention is over tokens 0..3 regardless of g.
 - softmax rows sum to 1 -> bv passes through attention; fold into
   bo_eff = bv@Wo + bo and drop from the V projection.
 - 0.5 local/global averaging folds into Wo (Wo_eff = 0.5*Wo).
 - attention scale folds into the Q projection epilogue.

v2 redesign (vs v1):
 - all matmuls in bf16 (fp32 runs 4 cycles/row via LOW_HIGH dual pass;
   bf16 runs 1).
 - scores computed TRANSPOSED: scT[k,q] = kT.T @ qz, so softmax sits
   along the partition (key) dim. No max subtraction is needed (scores
   are O(1) by construction), exp needs no accumulators, and the P
   transposes + gpsimd row-normalize of v1 disappear entirely.
 - head-pair batching: qz holds queries zero-interleaved per e-block
   ([qA;0] cols 0..512, [0;qB] cols 512..1024), so one 128-partition
   matmul computes scores for two heads without cross-head mixing.
 - V carries an appended ones column per head (vvx layout 12*65), so
   the PV matmul emits the softmax denominators for free (out[q,64]).
 - PV output is [q, d]: denominators land per-partition, so the
   normalize is a cheap per-partition activation scale + DVE fused
   scalar_tensor_tensor combine; o is then PE-transposed (24 [128,128]
   transposes) into [emb, tok] for the output projection.
"""

import sys

if "/opt/trn_rl_repo" not in sys.path:
    sys.path.insert(0, "/opt/trn_rl_repo")

import numpy as np

B = 2
S = 2048
DIM = 768
HEADS = 12
D = 64
W2 = 32  # half window
NCHUNK = 4
CHUNK = S // NCHUNK  # 512
HALO = CHUNK + 2 * W2  # 576
NTOK = 640  # 576 halo'd tokens + 4 global + 60 zero pad
NG = 4
QB = 128  # query block
NQB = CHUNK // QB  # 4
FB = 6  # 768 / 128 blocks
VW = HEADS * (D + 1)  # 780: V block width with ones col per head
SCW = 512  # score tile cols: (qA|qB) x (Akeys | B+G keys)
SCALE = D ** -0.5

_STATE: dict = {}


def _build_bass():
    from contextlib import ExitStack

    import concourse.bass as bass  # noqa: F401
    import concourse.mybir as mybir
    import concourse.tile as tile
    from concourse import bacc
    from concourse.masks import make_identity

    f32 = mybir.dt.float32
    bf16 = mybir.dt.bfloat16
    AF = mybir.ActivationFunctionType
    OP = mybir.AluOpType

    nc = bacc.Bacc("TRN2", target_bir_lowering=False)

    # xw packs [xT_f 640 | Wq_f 768] per 128-row block: one DMA per f-block
    # lets the Q projection start ~2us in. consts packs biases + mask.
    XWW = NTOK + DIM  # 1408
    xw_d = nc.declare_dram_parameter("xw", [DIM, XWW], bf16, isOutput=False)
    wk_d = nc.declare_dram_parameter("Wk", [DIM, DIM], bf16, isOutput=False)
    wv_d = nc.declare_dram_parameter("Wv", [DIM, DIM], bf16, isOutput=False)
    wo_d = nc.declare_dram_parameter("Wo", [DIM, DIM], bf16, isOutput=False)
    CW = 3 * FB  # 18
    consts_d = nc.declare_dram_parameter("consts", [128, CW], f32, isOutput=False)
    mask_d = nc.declare_dram_parameter("maskb", [128, NQB * 2 * SCW], bf16, isOutput=False)
    outT_d = nc.declare_dram_parameter("outT", [DIM, CHUNK], bf16, isOutput=True)

    with tile.TileContext(nc) as tc, ExitStack() as ctx:
        const = ctx.enter_context(tc.tile_pool(name="const", bufs=1))
        big = ctx.enter_context(tc.tile_pool(name="big", bufs=1))
        work = ctx.enter_context(tc.tile_pool(name="work", bufs=4))
        small = ctx.enter_context(tc.tile_pool(name="small", bufs=6))

        ident = const.tile([128, 128], bf16, tag="ident")
        make_identity(nc, ident[:])

        # DMA order: consts (epilogues need biases), then xw per f-block
        # (Q proj chases the stream), then wk/wv/wo ordered by first use.
        consts = const.tile([128, CW], f32, tag="consts")
        nc.sync.dma_start(out=consts[:], in_=consts_d[:, :])
        bq2 = consts[:, 0:FB]
        bk2 = consts[:, FB : 2 * FB]
        bo2 = consts[:, 2 * FB : 3 * FB]
        maskt = const.tile([128, NQB * 2 * SCW], bf16, tag="maskb")

        xw = big.tile([128, FB * XWW], bf16, tag="xw")
        for f in range(FB):
            nc.sync.dma_start(
                out=xw[:, f * XWW : (f + 1) * XWW],
                in_=xw_d[f * 128 : (f + 1) * 128, :],
            )
        wk = big.tile([128, FB * DIM], bf16, tag="wk")
        nc.sync.dma_start(
            out=wk[:].rearrange("p (f c) -> p f c", f=FB),
            in_=wk_d[:, :].rearrange("(f p) c -> p f c", p=128),
        )
        nc.sync.dma_start(out=maskt[:], in_=mask_d[:, :])
        wv = big.tile([128, FB * DIM], bf16, tag="wv")
        nc.sync.dma_start(
            out=wv[:].rearrange("p (f c) -> p f c", f=FB),
            in_=wv_d[:, :].rearrange("(f p) c -> p f c", p=128),
        )
        wo = big.tile([128, FB * DIM], bf16, tag="wo")
        nc.sync.dma_start(
            out=wo[:].rearrange("p (f c) -> p f c", f=FB),
            in_=wo_d[:, :].rearrange("(f p) c -> p f c", p=128),
        )

        # qz: per e-block [128, 1024]: cols 0..512 head A (rows 0..64,
        # zeros below), cols 512..1024 head B (rows 64..128, zeros above)
        qz = big.tile([128, FB * 1024], bf16, tag="qz")
        qz_v = qz[:].rearrange("p (e c) -> p e c", e=FB)
        nc.vector.memset(qz_v[0:64, :, 512:1024], 0.0)
        nc.vector.memset(qz_v[64:128, :, 0:512], 0.0)

        kT = big.tile([128, FB * NTOK], bf16, tag="kT")
        # vvx: token-major V, 5 blocks of [128 tok, 12 heads x (64 d + one)]
        vvx = big.tile([128, 5 * VW], bf16, tag="vvx")
        nc.vector.memset(
            vvx[:].rearrange("p (t h c) -> p (t h) c", t=5, h=HEADS)[:, :, D : D + 1],
            1.0,
        )
        oT = big.tile([128, FB * CHUNK], bf16, tag="oT")
        outT = big.tile([128, FB * CHUNK], bf16, tag="outT")

        # ---- Compute: projections interleaved with attention ----
        # All PSUM pools coexist (2+3+2+1 = 8 banks); projection work for
        # later-needed K/V blocks is emitted BETWEEN early attention duos so
        # projection matmuls fill the PE while attention chains stall.
        P = {}
        scratch = work.tile([128, 512], bf16, tag="scratch")
        nc.gpsimd.memset(scratch[:], 0.0)

        def emit_q(e):
            ps = P["proj"].tile([128, 512], f32, tag="proj")
            for f in range(FB):
                nc.tensor.matmul(
                    ps[:],
                    xw[:, f * XWW + NTOK + e * 128 : f * XWW + NTOK + e * 128 + 128],
                    xw[:, f * XWW + W2 : f * XWW + W2 + CHUNK],
                    start=(f == 0),
                    stop=(f == FB - 1),
                )
            nc.scalar.activation(
                qz[0:64, e * 1024 : e * 1024 + 512],
                ps[0:64, :],
                AF.Identity,
                bias=bq2[0:64, e : e + 1],
                scale=SCALE,
            )
            nc.scalar.activation(
                qz[64:128, e * 1024 + 512 : e * 1024 + 1024],
                ps[64:128, :],
                AF.Identity,
                bias=bq2[64:128, e : e + 1],
                scale=SCALE,
            )

        def emit_k(e):
            for c0, w in ((0, 512), (512, 128)):
                ps = P["proj"].tile([128, w], f32, tag="proj")
                for f in range(FB):
                    nc.tensor.matmul(
                        ps[:],
                        wk[:, f * DIM + e * 128 : f * DIM + e * 128 + 128],
                        xw[:, f * XWW + c0 : f * XWW + c0 + w],
                        start=(f == 0),
                        stop=(f == FB - 1),
                    )
                nc.vector.tensor_scalar_add(
                    kT[:, e * NTOK + c0 : e * NTOK + c0 + w],
                    ps[:],
                    bk2[:, e : e + 1],
                )

        def emit_v(t):
            for c0, w in ((0, 512), (512, 256)):
                ps = P["proj"].tile([128, w], f32, tag="proj")
                for f in range(FB):
                    nc.tensor.matmul(
                        ps[:],
                        xw[:, f * XWW + t * 128 : f * XWW + t * 128 + 128],
                        wv[:, f * DIM + c0 : f * DIM + c0 + w],
                        start=(f == 0),
                        stop=(f == FB - 1),
                    )
                dest = vvx[:, t * VW : (t + 1) * VW].rearrange(
                    "p (h c) -> p h c", c=D + 1
                )[:, c0 // D : (c0 + w) // D, 0:D]
                nc.scalar.activation(dest, ps[:], AF.Identity, bias=0.0, scale=1.0)

        # global-token V rows (tokens 0..3 live at rows 64..68 of block 4);
        # staged at partitions 64..68 so the PV matmul shares prT's base
        # partition (lhsT/rhs must have equal base partitions).
        vgx = const.tile([128, VW], bf16, tag="vgx")

        def emit_duo(qb, e2, o_qb):
            prT2 = work.tile([128, 2 * SCW], bf16, tag="prT")
            for de in range(2):
                e = e2 + de
                qsl = qz[:, e * 1024 : (e + 1) * 1024].rearrange(
                    "p (s q) -> p s q", s=2
                )[:, :, qb * QB : (qb + 1) * QB]

                sc = P["s"].tile([128, SCW], f32, tag="sc")
                nc.tensor.matmul(
                    sc[:, 0:256],
                    kT[:, e * NTOK + qb * QB : e * NTOK + qb * QB + 128],
                    qsl,
                    start=True,
                    stop=True,
                )
                nc.tensor.matmul(
                    sc[0:64, 256:512],
                    kT[:, e * NTOK + qb * QB + 128 : e * NTOK + qb * QB + 192],
                    qsl,
                    start=True,
                    stop=True,
                )
                nc.tensor.matmul(
                    sc[64:128, 256:512],
                    kT[:, e * NTOK + HALO : e * NTOK + NTOK],
                    qsl,
                    start=True,
                    stop=True,
                )
                # exp (scores are O(1): no max subtraction needed)
                nc.scalar.activation(
                    prT2[:, de * SCW : (de + 1) * SCW],
                    sc[:],
                    AF.Exp,
                    bias=0.0,
                    scale=1.0,
                )
                # multiplicative 0/1 band mask, split across gpsimd and DVE
                # so the two pairs' masks run concurrently and each pair's
                # PV unblocks as soon as its own half is masked
                eng = nc.vector if de == 0 else nc.gpsimd
                eng.tensor_mul(
                    prT2[:, de * SCW : (de + 1) * SCW],
                    prT2[:, de * SCW : (de + 1) * SCW],
                    maskt[:, qb * 2 * SCW + de * SCW : qb * 2 * SCW + (de + 1) * SCW],
                )

            for de in range(2):
                e = e2 + de
                hA, hB = 2 * e, 2 * e + 1
                pb = de * SCW
                # PV: out[q, d], denominators in col 64 of each 65-group
                ot = P["ot"].tile([128, 4 * (D + 1)], f32, tag="ot")
                for i, h in ((0, hA), (2, hB)):
                    qc = pb + (0 if h == hA else 128)
                    nc.tensor.matmul(
                        ot[:, i * (D + 1) : (i + 1) * (D + 1)],
                        prT2[:, qc : qc + 128],
                        vvx[:, qb * VW + h * (D + 1) : qb * VW + (h + 1) * (D + 1)],
                        start=True,
                        stop=False,
                    )
                    nc.tensor.matmul(
                        ot[:, i * (D + 1) : (i + 1) * (D + 1)],
                        prT2[0:64, 256 + qc : 256 + qc + 128],
                        vvx[
                            0:64,
                            (qb + 1) * VW + h * (D + 1) : (qb + 1) * VW
                            + (h + 1) * (D + 1),
                        ],
                        start=False,
                        stop=True,
                    )
                    nc.tensor.matmul(
                        ot[:, (i + 1) * (D + 1) : (i + 2) * (D + 1)],
                        prT2[64:68, 256 + qc : 256 + qc + 128],
                        vgx[64:68, h * (D + 1) : (h + 1) * (D + 1)],
                        start=True,
                        stop=True,
                    )

                # normalize+combine: o = ul/sl + ug/sg (0.5 folded in Wo)
                rr = small.tile([128, 4], f32, tag="rr")
                nc.vector.reciprocal(
                    rr[:],
                    ot[:].rearrange("p (h c) -> p h c", c=D + 1)[:, :, D : D + 1],
                )
                for i, h in ((0, hA), (2, hB)):
                    tg = small.tile([128, D], f32, tag="tg")
                    nc.scalar.activation(
                        tg[:],
                        ot[:, (i + 1) * (D + 1) : (i + 1) * (D + 1) + D],
                        AF.Identity,
                        bias=0.0,
                        scale=rr[:, i + 1 : i + 2],
                    )
                    nc.vector.scalar_tensor_tensor(
                        o_qb[:, h * D : (h + 1) * D],
                        ot[:, i * (D + 1) : i * (D + 1) + D],
                        rr[:, i : i + 1],
                        tg[:],
                        OP.mult,
                        OP.add,
                    )

        def emit_tr(qb, o_qb):
            # transpose o_qb [q, emb] -> oT [emb, q-cols]
            for c in range(FB):
                pt = P["tr"].tile([128, 128], bf16, tag="pt")
                nc.tensor.transpose(pt[:], o_qb[:, c * 128 : (c + 1) * 128], ident[:])
                if c % 2 == 0:
                    nc.scalar.copy(
                        oT[:, c * CHUNK + qb * QB : c * CHUNK + (qb + 1) * QB], pt[:]
                    )
                else:
                    nc.vector.tensor_copy(
                        oT[:, c * CHUNK + qb * QB : c * CHUNK + (qb + 1) * QB], pt[:]
                    )

        # Emission schedule: phase-scoped PSUM pools (projections get 4
        # banks of pipelining; attention then re-scopes to 3+3+2).
        proj_ctx = ExitStack()
        P["proj"] = proj_ctx.enter_context(
            tc.tile_pool(name="pp_proj", bufs=4, space="PSUM")
        )
        # HAM warm-up spin: the PE clock-gate defaults to half rate and
        # releases after ~3.4us of sustained activity. Burn wide matmuls
        # (inputs on-chip) while the input DMAs stream so projections
        # start warm.
        for _ in range(9):
            sp = P["proj"].tile([128, 512], f32, tag="proj", name="spin")
            nc.tensor.matmul(sp[:], ident[:], scratch[:], start=True, stop=True)
        for e in range(FB):
            emit_q(e)
        for e in range(FB):
            emit_k(e)
        for t in range(5):
            emit_v(t)
        nc.sync.dma_start(out=vgx[64:68, :], in_=vvx[64:68, 4 * VW : 5 * VW])

        proj_ctx.close()
        attn_ctx = ExitStack()
        P["s"] = attn_ctx.enter_context(
            tc.tile_pool(name="pp_s", bufs=4, space="PSUM")
        )
        P["ot"] = attn_ctx.enter_context(
            tc.tile_pool(name="pp_ot", bufs=2, space="PSUM")
        )
        P["tr"] = attn_ctx.enter_context(
            tc.tile_pool(name="pp_tr", bufs=2, space="PSUM")
        )
        for qb in range(NQB):
            o_qb = work.tile([128, DIM], bf16, tag="oqb")
            for e2 in range(0, FB, 2):
                emit_duo(qb, e2, o_qb)
            emit_tr(qb, o_qb)

        attn_ctx.close()
        P["proj"] = ctx.enter_context(tc.tile_pool(name="pp_o", bufs=4, space="PSUM"))
        for _ in range(8):
            sp = P["proj"].tile([128, 512], f32, tag="proj", name="spin2")
            nc.tensor.matmul(sp[:], ident[:], scratch[:], start=True, stop=True)


        # ---- Output projection ----
        # Output projection: outT[e,t] = sum_c Wo_eff[c,e] oT[c,t] + bo_eff
        for e in range(FB):
            ps = P["proj"].tile([128, 512], f32, tag="proj", name="ops")
            for c in range(FB):
                nc.tensor.matmul(
                    ps[:],
                    wo[:, c * DIM + e * 128 : c * DIM + e * 128 + 128],
                    oT[:, c * CHUNK : (c + 1) * CHUNK],
                    start=(c == 0),
                    stop=(c == FB - 1),
                )
            nc.scalar.activation(
                outT[:, e * CHUNK : (e + 1) * CHUNK],
                ps[:],
                AF.Identity,
                bias=bo2[:, e : e + 1],
                scale=1.0,
            )
        nc.sync.dma_start(
            out=outT_d[0 : 5 * 128, :].rearrange("(f p) c -> p f c", p=128),
            in_=outT[:, 0 : 5 * CHUNK].rearrange("p (f c) -> p f c", f=5),
        )
        nc.sync.dma_start(
            out=outT_d[5 * 128 : 6 * 128, :],
            in_=outT[:, 5 * CHUNK : 6 * CHUNK],
        )

    if not nc.is_finalized():
        nc.finalize()
    return nc


def _get_nc():
    if "nc" not in _STATE:
        _STATE["nc"] = _build_bass()
    return _STATE["nc"]


def _host_masks():
    # 0/1 keep-mask, bf16: [128 keys, NQB*512]: per qb, cols 0..256 are
    # A-block keys x (qA|qB), cols 256..512 are B+global keys x (qA|qB).
    if "masks" in _STATE:
        return _STATE["masks"]
    import ml_dtypes

    masks = []
    q = np.arange(QB)[None, :]  # query within block (free dim)
    kA = np.arange(128)[:, None]  # A-block key partition
    kB = np.arange(64)[:, None]  # B-block key partition
    for j in range(NCHUNK):
        m = np.zeros((128, NQB * SCW), np.float32)
        for qb in range(NQB):
            base = j * CHUNK + qb * QB - W2  # global token of halo key 0
            keepA = (np.abs(q + W2 - kA) <= W2) & (base + kA >= 0) & (base + kA < S)
            keepB = (
                (np.abs(q + W2 - (kB + 128)) <= W2)
                & (base + 128 + kB >= 0)
                & (base + 128 + kB < S)
            )
            blk = np.zeros((128, SCW), np.float32)
            mA = keepA.astype(np.float32)
            mB = keepB.astype(np.float32)
            blk[:, 0:128] = mA
            blk[:, 128:256] = mA
            blk[0:64, 256:384] = mB
            blk[0:64, 384:512] = mB
            blk[64:68, 256:512] = 1.0  # global keys unmasked
            m[:, qb * SCW : (qb + 1) * SCW] = blk
        m2 = m.reshape(128, NQB, SCW)
        m2 = np.concatenate([m2, m2], axis=2).reshape(128, NQB * 2 * SCW)
        masks.append(np.ascontiguousarray(m2.astype(ml_dtypes.bfloat16)))
    _STATE["masks"] = masks
    return masks


def kernel(x, Wq, bq, Wk, bk, Wv, bv, Wo, bo, g):
    import ml_dtypes
    from concourse.bass_utils import run_bass_kernel_spmd

    bf16 = ml_dtypes.bfloat16
    x = np.asarray(x, np.float32)
    Wq = np.ascontiguousarray(np.asarray(Wq, np.float32).astype(bf16))
    Wk = np.ascontiguousarray(np.asarray(Wk, np.float32).astype(bf16))
    Wv = np.ascontiguousarray(np.asarray(Wv, np.float32).astype(bf16))
    Wo_f = np.asarray(Wo, np.float32)
    Wo_eff = np.ascontiguousarray((0.5 * Wo_f).astype(bf16))
    bq = np.asarray(bq, np.float32)
    bk = np.asarray(bk, np.float32)
    bv = np.asarray(bv, np.float32)
    bo = np.asarray(bo, np.float32)
    # g unused: top_k over all 4 elements + permutation invariance of
    # attention means global attention is over tokens 0..3 regardless of g.

    bo_eff = bv @ Wo_f + bo
    bq2 = np.ascontiguousarray((bq * SCALE).reshape(FB, 128).T)
    bk2 = np.ascontiguousarray(bk.reshape(FB, 128).T)
    bo2 = np.ascontiguousarray(bo_eff.reshape(FB, 128).T)
    masks = _host_masks()

    in_maps = []
    consts_cache = {}
    for c in range(8):
        b, j = divmod(c, NCHUNK)
        xT = np.zeros((DIM, NTOK), np.float32)
        p_lo = W2 if j == 0 else 0
        p_hi = HALO - W2 if j == NCHUNK - 1 else HALO
        r_lo = j * CHUNK - W2 + p_lo
        r_hi = j * CHUNK - W2 + p_hi
        xT[:, p_lo:p_hi] = x[b, r_lo:r_hi, :].T
        xT[:, HALO : HALO + NG] = x[b, 0:NG, :].T
        xw = np.concatenate([xT.astype(bf16), Wq], axis=1)
        if "consts" not in consts_cache:
            consts_cache["consts"] = np.ascontiguousarray(
                np.concatenate([bq2, bk2, bo2], axis=1)
            )
        in_maps.append(
            {
                "xw": np.ascontiguousarray(xw),
                "Wk": Wk,
                "Wv": Wv,
                "Wo": Wo_eff,
                "consts": consts_cache["consts"],
                "maskb": masks[j],
            }
        )

    nc = _get_nc()
    res = run_bass_kernel_spmd(nc, in_maps, core_ids=list(range(8)))
    _STATE["last_results"] = res

    out = np.empty((B, S, DIM), np.float32)
    for c in range(8):
        b, j = divmod(c, NCHUNK)
        out[b, j * CHUNK : (j + 1) * CHUNK, :] = (
            res.results[c]["outT"].astype(np.float32).T
        )
    return out


# revision 55
# speedup vs baseline: 1.0200x; 1.0200x over previous
"""LocalGlobalAttention Bass/Tile kernel for 8 Trainium2 NeuronCores.

Sharding: data-parallel over (batch=2) x (4 sequence chunks of 512).
Each core independently computes QKV projection (with +-32 token halo),
banded local attention (|i-j|<=32), global attention over tokens 0..3,
and the output projection for its 512-token slice. No collectives.

Exact-math host-side folds (same as v1):
 - top_k(softmax(g), 4) over a length-4 g selects all 4 indices ->
   global attention is over tokens 0..3 regardless of g.
 - softmax rows sum to 1 -> bv passes through attention; fold into
   bo_eff = bv@Wo + bo and drop from the V projection.
 - 0.5 local/global averaging folds into Wo (Wo_eff = 0.5*Wo).
 - attention scale folds into the Q projection epilogue.

v2 redesign (vs v1):
 - all matmuls in bf16 (fp32 runs 4 cycles/row via LOW_HIGH dual pass;
   bf16 runs 1).
 - scores computed TRANSPOSED: scT[k,q] = kT.T @ qz, so softmax sits
   along the partition (key) dim. No max subtraction is needed (scores
   are O(1) by construction), exp needs no accumulators, and the P
   transposes + gpsimd row-normalize of v1 disappear entirely.
 - head-pair batching: qz holds queries zero-interleaved per e-block
   ([qA;0] cols 0..512, [0;qB] cols 512..1024), so one 128-partition
   matmul computes scores for two heads without cross-head mixing.
 - V carries an appended ones column per head (vvx layout 12*65), so
   the PV matmul emits the softmax denominators for free (out[q,64]).
 - PV output is [q, d]: denominators land per-partition, so the
   normalize is a cheap per-partition activation scale + DVE fused
   scalar_tensor_tensor combine; o is then PE-transposed (24 [128,128]
   transposes) into [emb, tok] for the output projection.
"""

import sys

if "/opt/trn_rl_repo" not in sys.path:
    sys.path.insert(0, "/opt/trn_rl_repo")

import numpy as np

B = 2
S = 2048
DIM = 768
HEADS = 12
D = 64
W2 = 32  # half window
NCHUNK = 4
CHUNK = S // NCHUNK  # 512
HALO = CHUNK + 2 * W2  # 576
NTOK = 640  # 576 halo'd tokens + 4 global + 60 zero pad
NG = 4
QB = 128  # query block
NQB = CHUNK // QB  # 4
FB = 6  # 768 / 128 blocks
VW = HEADS * (D + 1)  # 780: V block width with ones col per head
SCW = 512  # score tile cols: (qA|qB) x (Akeys | B+G keys)
SCALE = D ** -0.5

_STATE: dict = {}


def _build_bass():
    from contextlib import ExitStack

    import concourse.bass as bass  # noqa: F401
    import concourse.mybir as mybir
    import concourse.tile as tile
    from concourse import bacc
    from concourse.masks import make_identity

    f32 = mybir.dt.float32
    bf16 = mybir.dt.bfloat16
    AF = mybir.ActivationFunctionType
    OP = mybir.AluOpType

    nc = bacc.Bacc("TRN2", target_bir_lowering=False)

    # xw packs [xT_f 640 | Wq_f 768] per 128-row block: one DMA per f-block
    # lets the Q projection start ~2us in. consts packs biases + mask.
    XWW = NTOK + DIM  # 1408
    xw_d = nc.declare_dram_parameter("xw", [DIM, XWW], bf16, isOutput=False)
    wk_d = nc.declare_dram_parameter("Wk", [DIM, DIM], bf16, isOutput=False)
    wv_d = nc.declare_dram_parameter("Wv", [DIM, DIM], bf16, isOutput=False)
    wo_d = nc.declare_dram_parameter("Wo", [DIM, DIM], bf16, isOutput=False)
    CW = 3 * FB  # 18
    consts_d = nc.declare_dram_parameter("consts", [128, CW], f32, isOutput=False)
    mask_d = nc.declare_dram_parameter("maskb", [128, NQB * 2 * SCW], bf16, isOutput=False)
    outT_d = nc.declare_dram_parameter("outT", [DIM, CHUNK], bf16, isOutput=True)

    with tile.TileContext(nc) as tc, ExitStack() as ctx:
        const = ctx.enter_context(tc.tile_pool(name="const", bufs=1))
        big = ctx.enter_context(tc.tile_pool(name="big", bufs=1))
        work = ctx.enter_context(tc.tile_pool(name="work", bufs=4))
        small = ctx.enter_context(tc.tile_pool(name="small", bufs=6))

        ident = const.tile([128, 128], bf16, tag="ident")
        make_identity(nc, ident[:])

        # DMA order: consts (epilogues need biases), then xw per f-block
        # (Q proj chases the stream), then wk/wv/wo ordered by first use.
        consts = const.tile([128, CW], f32, tag="consts")
        nc.sync.dma_start(out=consts[:], in_=consts_d[:, :])
        bq2 = consts[:, 0:FB]
        bk2 = consts[:, FB : 2 * FB]
        bo2 = consts[:, 2 * FB : 3 * FB]
        maskt = const.tile([128, NQB * 2 * SCW], bf16, tag="maskb")

        xw = big.tile([128, FB * XWW], bf16, tag="xw")
        for f in range(FB):
            nc.sync.dma_start(
                out=xw[:, f * XWW : (f + 1) * XWW],
                in_=xw_d[f * 128 : (f + 1) * 128, :],
            )
        wk = big.tile([128, FB * DIM], bf16, tag="wk")
        nc.sync.dma_start(
            out=wk[:].rearrange("p (f c) -> p f c", f=FB),
            in_=wk_d[:, :].rearrange("(f p) c -> p f c", p=128),
        )
        nc.sync.dma_start(out=maskt[:], in_=mask_d[:, :])
        wv = big.tile([128, FB * DIM], bf16, tag="wv")
        nc.sync.dma_start(
            out=wv[:].rearrange("p (f c) -> p f c", f=FB),
            in_=wv_d[:, :].rearrange("(f p) c -> p f c", p=128),
        )
        wo = big.tile([128, FB * DIM], bf16, tag="wo")
        nc.sync.dma_start(
            out=wo[:].rearrange("p (f c) -> p f c", f=FB),
            in_=wo_d[:, :].rearrange("(f p) c -> p f c", p=128),
        )

        # qz: per e-block [128, 1024]: cols 0..512 head A (rows 0..64,
        # zeros below), cols 512..1024 head B (rows 64..128, zeros above)
        qz = big.tile([128, FB * 1024], bf16, tag="qz")
        qz_v = qz[:].rearrange("p (e c) -> p e c", e=FB)
        nc.vector.memset(qz_v[0:64, :, 512:1024], 0.0)
        nc.vector.memset(qz_v[64:128, :, 0:512], 0.0)

        kT = big.tile([128, FB * NTOK], bf16, tag="kT")
        # vvx: token-major V, 5 blocks of [128 tok, 12 heads x (64 d + one)]
        vvx = big.tile([128, 5 * VW], bf16, tag="vvx")
        nc.vector.memset(
            vvx[:].rearrange("p (t h c) -> p (t h) c", t=5, h=HEADS)[:, :, D : D + 1],
            1.0,
        )
        oT = big.tile([128, FB * CHUNK], bf16, tag="oT")
        outT = big.tile([128, FB * CHUNK], bf16, tag="outT")

        # ---- Compute: projections interleaved with attention ----
        # All PSUM pools coexist (2+3+2+1 = 8 banks); projection work for
        # later-needed K/V blocks is emitted BETWEEN early attention duos so
        # projection matmuls fill the PE while attention chains stall.
        P = {}
        scratch = work.tile([128, 512], bf16, tag="scratch")
        nc.gpsimd.memset(scratch[:], 0.0)

        def emit_q(e):
            ps = P["proj"].tile([128, 512], f32, tag="proj")
            for f in range(FB):
                nc.tensor.matmul(
                    ps[:],
                    xw[:, f * XWW + NTOK + e * 128 : f * XWW + NTOK + e * 128 + 128],
                    xw[:, f * XWW + W2 : f * XWW + W2 + CHUNK],
                    start=(f == 0),
                    stop=(f == FB - 1),
                )
            nc.scalar.activation(
                qz[0:64, e * 1024 : e * 1024 + 512],
                ps[0:64, :],
                AF.Identity,
                bias=bq2[0:64, e : e + 1],
                scale=SCALE,
            )
            nc.scalar.activation(
                qz[64:128, e * 1024 + 512 : e * 1024 + 1024],
                ps[64:128, :],
                AF.Identity,
                bias=bq2[64:128, e : e + 1],
                scale=SCALE,
            )

        def emit_k(e):
            for c0, w in ((0, 512), (512, 128)):
                ps = P["proj"].tile([128, w], f32, tag="proj")
                for f in range(FB):
                    nc.tensor.matmul(
                        ps[:],
                        wk[:, f * DIM + e * 128 : f * DIM + e * 128 + 128],
                        xw[:, f * XWW + c0 : f * XWW + c0 + w],
                        start=(f == 0),
                        stop=(f == FB - 1),
                    )
                nc.vector.tensor_scalar_add(
                    kT[:, e * NTOK + c0 : e * NTOK + c0 + w],
                    ps[:],
                    bk2[:, e : e + 1],
                )

        def emit_v(t):
            for c0, w in ((0, 512), (512, 256)):
                ps = P["proj"].tile([128, w], f32, tag="proj")
                for f in range(FB):
                    nc.tensor.matmul(
                        ps[:],
                        xw[:, f * XWW + t * 128 : f * XWW + t * 128 + 128],
                        wv[:, f * DIM + c0 : f * DIM + c0 + w],
                        start=(f == 0),
                        stop=(f == FB - 1),
                    )
                dest = vvx[:, t * VW : (t + 1) * VW].rearrange(
                    "p (h c) -> p h c", c=D + 1
                )[:, c0 // D : (c0 + w) // D, 0:D]
                nc.scalar.activation(dest, ps[:], AF.Identity, bias=0.0, scale=1.0)

        # global-token V rows (tokens 0..3 live at rows 64..68 of block 4);
        # staged at partitions 64..68 so the PV matmul shares prT's base
        # partition (lhsT/rhs must have equal base partitions).
        vgx = const.tile([128, VW], bf16, tag="vgx")

        def emit_duo(qb, e2, o_qb):
            prT2 = work.tile([128, 2 * SCW], bf16, tag="prT")
            for de in range(2):
                e = e2 + de
                qsl = qz[:, e * 1024 : (e + 1) * 1024].rearrange(
                    "p (s q) -> p s q", s=2
                )[:, :, qb * QB : (qb + 1) * QB]

                sc = P["s"].tile([128, SCW], f32, tag="sc")
                nc.tensor.matmul(
                    sc[:, 0:256],
                    kT[:, e * NTOK + qb * QB : e * NTOK + qb * QB + 128],
                    qsl,
                    start=True,
                    stop=True,
                )
                nc.tensor.matmul(
                    sc[0:64, 256:512],
                    kT[:, e * NTOK + qb * QB + 128 : e * NTOK + qb * QB + 192],
                    qsl,
                    start=True,
                    stop=True,
                )
                nc.tensor.matmul(
                    sc[64:128, 256:512],
                    kT[:, e * NTOK + HALO : e * NTOK + NTOK],
                    qsl,
                    start=True,
                    stop=True,
                )
                # exp (scores are O(1): no max subtraction needed)
                nc.scalar.activation(
                    prT2[:, de * SCW : (de + 1) * SCW],
                    sc[:],
                    AF.Exp,
                    bias=0.0,
                    scale=1.0,
                )
                # multiplicative 0/1 band mask, split across gpsimd and DVE
                # so the two pairs' masks run concurrently and each pair's
                # PV unblocks as soon as its own half is masked
                eng = nc.vector if de == 0 else nc.gpsimd
                eng.tensor_mul(
                    prT2[:, de * SCW : (de + 1) * SCW],
                    prT2[:, de * SCW : (de + 1) * SCW],
                    maskt[:, qb * 2 * SCW + de * SCW : qb * 2 * SCW + (de + 1) * SCW],
                )

            for de in range(2):
                e = e2 + de
                hA, hB = 2 * e, 2 * e + 1
                pb = de * SCW
                # PV: out[q, d], denominators in col 64 of each 65-group
                ot = P["ot"].tile([128, 4 * (D + 1)], f32, tag="ot")
                for i, h in ((0, hA), (2, hB)):
                    qc = pb + (0 if h == hA else 128)
                    nc.tensor.matmul(
                        ot[:, i * (D + 1) : (i + 1) * (D + 1)],
                        prT2[:, qc : qc + 128],
                        vvx[:, qb * VW + h * (D + 1) : qb * VW + (h + 1) * (D + 1)],
                        start=True,
                        stop=False,
                    )
                    nc.tensor.matmul(
                        ot[:, i * (D + 1) : (i + 1) * (D + 1)],
                        prT2[0:64, 256 + qc : 256 + qc + 128],
                        vvx[
                            0:64,
                            (qb + 1) * VW + h * (D + 1) : (qb + 1) * VW
                            + (h + 1) * (D + 1),
                        ],
                        start=False,
                        stop=True,
                    )
                    nc.tensor.matmul(
                        ot[:, (i + 1) * (D + 1) : (i + 2) * (D + 1)],
                        prT2[64:68, 256 + qc : 256 + qc + 128],
                        vgx[64:68, h * (D + 1) : (h + 1) * (D + 1)],
                        start=True,
                        stop=True,
                    )

                # normalize+combine: o = ul/sl + ug/sg (0.5 folded in Wo)
                rr = small.tile([128, 4], f32, tag="rr")
                nc.vector.reciprocal(
                    rr[:],
                    ot[:].rearrange("p (h c) -> p h c", c=D + 1)[:, :, D : D + 1],
                )
                for i, h in ((0, hA), (2, hB)):
                    tg = small.tile([128, D], f32, tag="tg")
                    nc.scalar.activation(
                        tg[:],
                        ot[:, (i + 1) * (D + 1) : (i + 1) * (D + 1) + D],
                        AF.Identity,
                        bias=0.0,
                        scale=rr[:, i + 1 : i + 2],
                    )
                    nc.vector.scalar_tensor_tensor(
                        o_qb[:, h * D : (h + 1) * D],
                        ot[:, i * (D + 1) : i * (D + 1) + D],
                        rr[:, i : i + 1],
                        tg[:],
                        OP.mult,
                        OP.add,
                    )

        def emit_tr(qb, o_qb):
            # transpose o_qb [q, emb] -> oT [emb, q-cols]
            for c in range(FB):
                pt = P["tr"].tile([128, 128], bf16, tag="pt")
                nc.tensor.transpose(pt[:], o_qb[:, c * 128 : (c + 1) * 128], ident[:])
                if c % 2 == 0:
                    nc.scalar.copy(
                        oT[:, c * CHUNK + qb * QB : c * CHUNK + (qb + 1) * QB], pt[:]
                    )
                else:
                    nc.vector.tensor_copy(
                        oT[:, c * CHUNK + qb * QB : c * CHUNK + (qb + 1) * QB], pt[:]
                    )

        # Emission schedule: phase-scoped PSUM pools (projections get 4
        # banks of pipelining; attention then re-scopes to 3+3+2).
        proj_ctx = ExitStack()
        P["proj"] = proj_ctx.enter_context(
            tc.tile_pool(name="pp_proj", bufs=4, space="PSUM")
        )
        # HAM warm-up spin: the PE clock-gate defaults to half rate and
        # releases after ~3.4us of sustained activity. Burn wide matmuls
        # (inputs on-chip) while the input DMAs stream so projections
        # start warm.
        for _ in range(7):
            sp = P["proj"].tile([128, 512], f32, tag="proj", name="spin")
            nc.tensor.matmul(sp[:], ident[:], scratch[:], start=True, stop=True)
        for e in range(FB):
            emit_q(e)
        for e in range(FB):
            emit_k(e)
        for t in range(5):
            emit_v(t)
        nc.sync.dma_start(out=vgx[64:68, :], in_=vvx[64:68, 4 * VW : 5 * VW])

        proj_ctx.close()
        attn_ctx = ExitStack()
        P["s"] = attn_ctx.enter_context(
            tc.tile_pool(name="pp_s", bufs=4, space="PSUM")
        )
        P["ot"] = attn_ctx.enter_context(
            tc.tile_pool(name="pp_ot", bufs=2, space="PSUM")
        )
        P["tr"] = attn_ctx.enter_context(
            tc.tile_pool(name="pp_tr", bufs=2, space="PSUM")
        )
        for qb in range(NQB):
            o_qb = work.tile([128, DIM], bf16, tag="oqb")
            for e2 in range(0, FB, 2):
                emit_duo(qb, e2, o_qb)
            emit_tr(qb, o_qb)

        attn_ctx.close()
        P["proj"] = ctx.enter_context(tc.tile_pool(name="pp_o", bufs=4, space="PSUM"))
        for _ in range(8):
            sp = P["proj"].tile([128, 512], f32, tag="proj", name="spin2")
            nc.tensor.matmul(sp[:], ident[:], scratch[:], start=True, stop=True)


        # ---- Output projection ----
        # Output projection: outT[e,t] = sum_c Wo_eff[c,e] oT[c,t] + bo_eff
        for e in range(FB):
            ps = P["proj"].tile([128, 512], f32, tag="proj", name="ops")
            for c in range(FB):
                nc.tensor.matmul(
                    ps[:],
                    wo[:, c * DIM + e * 128 : c * DIM + e * 128 + 128],
                    oT[:, c * CHUNK : (c + 1) * CHUNK],
                    start=(c == 0),
                    stop=(c == FB - 1),
                )
            nc.scalar.activation(
                outT[:, e * CHUNK : (e + 1) * CHUNK],
                ps[:],
                AF.Identity,
                bias=bo2[:, e : e + 1],
                scale=1.0,
            )
        nc.sync.dma_start(
            out=outT_d[0 : 5 * 128, :].rearrange("(f p) c -> p f c", p=128),
            in_=outT[:, 0 : 5 * CHUNK].rearrange("p (f c) -> p f c", f=5),
        )
        nc.sync.dma_start(
            out=outT_d[5 * 128 : 6 * 128, :],
            in_=outT[:, 5 * CHUNK : 6 * CHUNK],
        )

    if not nc.is_finalized():
        nc.finalize()
    return nc


def _get_nc():
    if "nc" not in _STATE:
        _STATE["nc"] = _build_bass()
    return _STATE["nc"]


def _host_masks():
    # 0/1 keep-mask, bf16: [128 keys, NQB*512]: per qb, cols 0..256 are
    # A-block keys x (qA|qB), cols 256..512 are B+global keys x (qA|qB).
    if "masks" in _STATE:
        return _STATE["masks"]
    import ml_dtypes

    masks = []
    q = np.arange(QB)[None, :]  # query within block (free dim)
    kA = np.arange(128)[:, None]  # A-block key partition
    kB = np.arange(64)[:, None]  # B-block key partition
    for j in range(NCHUNK):
        m = np.zeros((128, NQB * SCW), np.float32)
        for qb in range(NQB):
            base = j * CHUNK + qb * QB - W2  # global token of halo key 0
            keepA = (np.abs(q + W2 - kA) <= W2) & (base + kA >= 0) & (base + kA < S)
            keepB = (
                (np.abs(q + W2 - (kB + 128)) <= W2)
                & (base + 128 + kB >= 0)
                & (base + 128 + kB < S)
            )
            blk = np.zeros((128, SCW), np.float32)
            mA = keepA.astype(np.float32)
            mB = keepB.astype(np.float32)
            blk[:, 0:128] = mA
            blk[:, 128:256] = mA
            blk[0:64, 256:384] = mB
            blk[0:64, 384:512] = mB
            blk[64:68, 256:512] = 1.0  # global keys unmasked
            m[:, qb * SCW : (qb + 1) * SCW] = blk
        m2 = m.reshape(128, NQB, SCW)
        m2 = np.concatenate([m2, m2], axis=2).reshape(128, NQB * 2 * SCW)
        masks.append(np.ascontiguousarray(m2.astype(ml_dtypes.bfloat16)))
    _STATE["masks"] = masks
    return masks


def kernel(x, Wq, bq, Wk, bk, Wv, bv, Wo, bo, g):
    import ml_dtypes
    from concourse.bass_utils import run_bass_kernel_spmd

    bf16 = ml_dtypes.bfloat16
    x = np.asarray(x, np.float32)
    Wq = np.ascontiguousarray(np.asarray(Wq, np.float32).astype(bf16))
    Wk = np.ascontiguousarray(np.asarray(Wk, np.float32).astype(bf16))
    Wv = np.ascontiguousarray(np.asarray(Wv, np.float32).astype(bf16))
    Wo_f = np.asarray(Wo, np.float32)
    Wo_eff = np.ascontiguousarray((0.5 * Wo_f).astype(bf16))
    bq = np.asarray(bq, np.float32)
    bk = np.asarray(bk, np.float32)
    bv = np.asarray(bv, np.float32)
    bo = np.asarray(bo, np.float32)
    # g unused: top_k over all 4 elements + permutation invariance of
    # attention means global attention is over tokens 0..3 regardless of g.

    bo_eff = bv @ Wo_f + bo
    bq2 = np.ascontiguousarray((bq * SCALE).reshape(FB, 128).T)
    bk2 = np.ascontiguousarray(bk.reshape(FB, 128).T)
    bo2 = np.ascontiguousarray(bo_eff.reshape(FB, 128).T)
    masks = _host_masks()

    in_maps = []
    consts_cache = {}
    for c in range(8):
        b, j = divmod(c, NCHUNK)
        xT = np.zeros((DIM, NTOK), np.float32)
        p_lo = W2 if j == 0 else 0
        p_hi = HALO - W2 if j == NCHUNK - 1 else HALO
        r_lo = j * CHUNK - W2 + p_lo
        r_hi = j * CHUNK - W2 + p_hi
        xT[:, p_lo:p_hi] = x[b, r_lo:r_hi, :].T
        xT[:, HALO : HALO + NG] = x[b, 0:NG, :].T
        xw = np.concatenate([xT.astype(bf16), Wq], axis=1)
        if "consts" not in consts_cache:
            consts_cache["consts"] = np.ascontiguousarray(
                np.concatenate([bq2, bk2, bo2], axis=1)
            )
        in_maps.append(
            {
                "xw": np.ascontiguousarray(xw),
                "Wk": Wk,
                "Wv": Wv,
                "Wo": Wo_eff,
                "consts": consts_cache["consts"],
                "maskb": masks[j],
            }
        )

    nc = _get_nc()
    res = run_bass_kernel_spmd(nc, in_maps, core_ids=list(range(8)))
    _STATE["last_results"] = res

    out = np.empty((B, S, DIM), np.float32)
    for c in range(8):
        b, j = divmod(c, NCHUNK)
        out[b, j * CHUNK : (j + 1) * CHUNK, :] = (
            res.results[c]["outT"].astype(np.float32).T
        )
    return out
